# revision 44
# baseline (speedup 1.0000x reference)
"""BitLinear (BitNet 1.58-bit ternary) distributed Trainium2 kernel.

Reference semantics:
    scale = max(mean(|w|), 1e-5)
    w_q   = sign(w) * (|w| > scale/3)          # ternary {-1, 0, 1}
    out   = (x @ w_q.T) * scale                # x: [4, 2048, 2048], w: [2048, 2048]

Sharding: data-parallel over tokens (1024 of 8192 per core), weight
replicated; each core computes the scale locally, so there are no
collectives (cross-core sync points absorb the harness' launch skew).

Host-side prep: transpose w to [in, out] and cast to fp16 — the ONLY
copy of w shipped (8 MiB/core instead of f32's 16+8), serving both the
scale reduction and the quantization.  fp16 rounding flips ~7e-5 of
the threshold comparisons (~0.9e-2 output rel err, gate is 2e-2).  x
is pre-cast to bf16 and pre-tiled m-major so every x DMA is contiguous
4KB-per-partition rows.

Pipeline (measured ~157us; baseline with f32-restream was ~209us):
  1. fp16 w streams once, 7x 1MiB + 3 smaller tail DMAs, on the SP
     HWDGE ring; |w| partial sums chase the stream, column-split
     between ACT (Abs + accum_out) and DVE (tensor_reduce absolute),
     whose combined throughput ~matches the ~460 GB/s arrival rate.
     x m-tiles queue behind w on the same FIFO ring — they land just
     before the matmul needs them without contending with the stream.
  2. scale: partials summed on DVE, broadcast to all 128 partitions
     with a ones-matmul into PSUM; thresholds +-t = +-scale/3 and the
     drain scale come straight off PSUM as dual-op tensor_scalars.
  3. quant, stored as 2*w_q (exact in bf16), split across engines at
     their measured rates (ACT sign 2.0us, DVE ~2.6us/tile for the
     3-op compare path):
       - 10 DVE-path tiles: 2[w>t] and -2[w<-t] via dual tensor_scalar
         (4x perf mode) + tensor_tensor subtract (2x);
       - 6 ACT-path tiles: Sign(w-t) written into wq, Sign(w+t) added
         by an SWDGE accumulate-DMA issued from the idle GpSimd queue
         (combine is off-engine; ~1.5 MiB SBUF traffic per tile rides
         unused fabric).
     The factor 2 folds into the drain scale (scale/2).
  4. matmul: bf16 PE, K contracted as 16 accumulating k-slices of 128.
     PSUM accumulation over k commutes, so phase 1 (m0,m1 over 7 PSUM
     banks, (m1,n3) deferred) consumes k-tiles in modeled production
     order, starting ~1.8us after the thresholds on a half-tile of k0.
     Warm-keeper dummy matmuls into the spare PSUM bank fill any
     production stall so the PE's HAM clock gate stays at 2.4 GHz (an
     idle 3.4us window would halve the clock and double the cost of
     the production-paced phase).  The deferred (m1,n3) pass then runs
     under the phase-1 drain copies, and phase 2 (m2..m7, n-outer /
     k-inner, 64 back-to-back matmuls per m-tile at the 216ns N=512
     roofline) streams out through per-n ACT copies (x scale/2) and
     output DMAs, the last one split so the final-byte tail is short.
"""
import sys

sys.path.insert(0, "/opt/trn_rl_repo")

import numpy as np

N_CORES = 8
B, S, D = 4, 2048, 2048        # x: [B, S, D]
OUT = 2048                     # out_features
TOK = B * S                    # 8192 tokens
TPC = TOK // N_CORES           # 1024 tokens per core
KT = D // 128                  # 16 K-tiles of 128
MT = TPC // 128                # 8 M-tiles per core
NT = OUT // 512                # 4 N-tiles of 512
N_ELEM = float(D * OUT)        # elements of w
EPS = 1e-5
M_P1 = 2                       # m-tiles in the k-outer first phase


def build_kernel():
    from concourse import bacc, tile, mybir
    from concourse.alu_op_type import AluOpType as Alu

    f32 = mybir.dt.float32
    bf16 = mybir.dt.bfloat16
    fp16 = mybir.dt.float16
    Act = mybir.ActivationFunctionType
    X = mybir.AxisListType.X

    nc = bacc.Bacc(None, target_bir_lowering=False)
    x_ext = nc.declare_dram_parameter("x", [TPC, D], bf16, isOutput=False)
    wh_ext = nc.declare_dram_parameter("wh", [D, OUT], fp16, isOutput=False)
    out_ext = nc.declare_dram_parameter("out", [TPC, OUT], f32, isOutput=True)

    with tile.TileContext(nc) as tc:
        with (
            tc.tile_pool(name="persist", bufs=1) as persist,
            tc.tile_pool(name="scr", bufs=1) as scr_pool,
            tc.tile_pool(name="sbuf2", bufs=4) as s_pool,
            tc.tile_pool(name="mbuf", bufs=2) as m_pool,
            tc.tile_pool(name="mpair", bufs=2) as mp_pool,
            tc.tile_pool(name="xbuf", bufs=8) as xbuf_pool,
            tc.tile_pool(name="stage", bufs=1) as stage_pool,
            tc.tile_pool(name="psum", bufs=8, space="PSUM") as psum_pool,
        ):
            wh = persist.tile([128, KT, OUT], fp16)      # resident fp16 w^T
            wq = persist.tile([128, KT, OUT], bf16)      # stored 2*w_q
            ones = persist.tile([128, 128], f32)
            partials = persist.tile([128, 30], f32)
            tot = persist.tile([128, 1], f32)
            tot2 = persist.tile([128, 1], f32)
            t_pos = persist.tile([128, 1], f32)
            t_neg = persist.tile([128, 1], f32)
            s_m = persist.tile([128, 1], f32)            # +scale/2
            scr = scr_pool.tile([128, 1968], fp16)       # |w| scratch (ACT Abs out)

            nc.vector.memset(ones[:], 1.0)
            nc.vector.memset(partials[:], 0.0)
            # pre-load the ACT function table so the first drain copy
            # doesn't pay ACT_TABLE_LOAD at phase-1 end
            tbl = persist.tile([128, 1], f32)
            nc.scalar.activation(tbl[:], ones[:, 0:1], Act.Copy)

            # ---- stream fp16 w: 7x 1MiB chunks + the last MiB as two
            # 0.5MiB DMAs (the second in two pieces) so the final
            # |w|-reduces start earlier.  ACT reduces most of the even
            # k-tile (Abs + accum_out), DVE its tail plus the odd tile
            # (tensor_reduce absolute) — an equal-lag column split at
            # the engines' measured rates. ----
            def reduce_pair(ke, ko, j):
                # ACT: 1968 cols of the even tile; DVE: its 80-col tail
                # plus the odd tile (equal-lag split at measured rates)
                nc.scalar.activation(
                    scr[:, 0:1968], wh[:, ke, 0:1968], Act.Abs,
                    accum_out=partials[:, j : j + 1],
                )
                nc.vector.tensor_reduce(
                    partials[:, 10 + j : 11 + j], wh[:, ke, 1968:],
                    axis=mybir.AxisListType.XY, op=Alu.add,
                    apply_absolute_value=True,
                )
                nc.vector.tensor_reduce(
                    partials[:, 20 + j : 21 + j], wh[:, ko, :],
                    axis=mybir.AxisListType.XY, op=Alu.add,
                    apply_absolute_value=True,
                )

            for j in range(KT // 2 - 1):
                nc.sync.dma_start(
                    wh[:, 2 * j : 2 * j + 2, :],
                    wh_ext[j * 256 : (j + 1) * 256, :].rearrange(
                        "(t p) o -> p t o", p=128
                    ),
                )
                reduce_pair(2 * j, 2 * j + 1, j)
            # last MiB as two half-chunks, each column-split across both
            nc.sync.dma_start(
                wh[:, KT - 2 : KT - 1, :],
                wh_ext[(KT - 2) * 128 : (KT - 1) * 128, :].rearrange(
                    "(t p) o -> p t o", p=128
                ),
            )
            nc.scalar.activation(
                scr[:, 0:1024], wh[:, KT - 2, 0:1024], Act.Abs,
                accum_out=partials[:, 7:8],
            )
            nc.vector.tensor_reduce(
                partials[:, 17:18], wh[:, KT - 2, 1024:],
                axis=mybir.AxisListType.XY, op=Alu.add,
                apply_absolute_value=True,
            )
            nc.sync.dma_start(
                wh[:, KT - 1 : KT, 0:1024],
                wh_ext[(KT - 1) * 128 :, 0:1024].rearrange(
                    "(t p) o -> p t o", p=128
                ),
            )
            nc.scalar.activation(
                scr[:, 0:1024], wh[:, KT - 1, 0:1024], Act.Abs,
                accum_out=partials[:, 8:9],
            )
            nc.sync.dma_start(
                wh[:, KT - 1 : KT, 1024:],
                wh_ext[(KT - 1) * 128 :, 1024:].rearrange(
                    "(t p) o -> p t o", p=128
                ),
            )
            nc.vector.tensor_reduce(
                partials[:, 18:19], wh[:, KT - 1, 1024:],
                axis=mybir.AxisListType.XY, op=Alu.add,
                apply_absolute_value=True,
            )

            # x m0/m1 queue on the SP ring behind the w stream, so they
            # land just before phase 1 without contending with it
            xbufs = {}

            def x_dma(m):
                xb = xbuf_pool.tile([128, KT, 128], bf16, tag="xbuf", name=f"xb{m}")
                nc.sync.dma_start(
                    xb[:],
                    x_ext[m * 128 : (m + 1) * 128, :].rearrange(
                        "p (k c) -> p k c", k=KT
                    ),
                )
                xbufs[m] = xb

            for m in range(MT):
                x_dma(m)

            # ---- PE warm-up: fp16 dummies gated on the last w chunk keep
            # the HAM busy so phase 1 starts at 2.4 GHz ----
            warm = psum_pool.tile([128, 512], f32, tag="psum", name="warm")
            for i in range(6):
                nc.tensor.matmul(
                    warm[:], wh[:, KT - 2, 0:128], wh[:, KT - 1, 0:512],
                    start=True, stop=True,
                )

            # ---- scale: sum partials, broadcast via ones-matmul ----
            nc.vector.tensor_reduce(tot[:], partials[:], axis=X, op=Alu.add)
            pbc = psum_pool.tile([128, 512], f32, tag="psum", name="pbc")
            nc.tensor.matmul(pbc[:, 0:1], ones[:], tot[:], start=True, stop=True)
            # keep PE busy through the scale->quant gap
            for i in range(4):
                nc.tensor.matmul(
                    warm[:], wh[:, KT - 2, 0:128], wh[:, KT - 1, 0:512],
                    start=True, stop=True,
                )
            nc.vector.tensor_scalar(
                t_pos[:], pbc[:, 0:1], 1.0 / (3.0 * N_ELEM), EPS / 3.0,
                Alu.mult, Alu.max,
            )
            nc.vector.tensor_scalar(
                t_neg[:], pbc[:, 0:1], -1.0 / (3.0 * N_ELEM), -EPS / 3.0,
                Alu.mult, Alu.min,
            )
            nc.vector.tensor_scalar(
                s_m[:], pbc[:, 0:1], 0.5 / N_ELEM, EPS / 2.0, Alu.mult, Alu.max,
            )

            # ---- quantize: stored wq = 2*w_q, split across all three
            # engines.  Odd k (ACT path): sign(w - t) + sign(w + t),
            # combined by GpSimd (k1..k9) or DVE (k11, k13); even k and
            # k15 (DVE path): 2[w > t] - 2[w < -t] via two dual
            # tensor_scalars (4x) and a tensor_tensor subtract (2x).
            # Drain scales by scale/2. ----
            def quantize(k, c0, c1):
                if k in (3, 4, 7, 8, 11, 12):
                    sn = s_pool.tile([128, OUT], bf16, tag="sbuf2", name=f"sn{k}")
                    nc.scalar.activation(
                        wq[:, k, c0:c1], wh[:, k, c0:c1], Act.Sign,
                        bias=t_neg[:, 0:1],
                    )
                    nc.scalar.activation(
                        sn[:, c0:c1], wh[:, k, c0:c1], Act.Sign, bias=t_pos[:, 0:1]
                    )
                    # combine off-engine: SWDGE accumulate-DMA adds sn into wq
                    nc.gpsimd.dma_start(
                        wq[:, k, c0:c1], sn[:, c0:c1], accum_op=Alu.add
                    )
                else:
                    a = m_pool.tile([128, OUT], bf16, tag="mbuf", name=f"a{k}_{c0}")
                    b = m_pool.tile([128, OUT], bf16, tag="mbuf", name=f"b{k}_{c0}")
                    nc.vector.tensor_scalar(
                        a[:, c0:c1], wh[:, k, c0:c1], t_pos[:, 0:1], 2.0,
                        Alu.is_gt, Alu.mult,
                    )
                    nc.vector.tensor_scalar(
                        b[:, c0:c1], wh[:, k, c0:c1], t_neg[:, 0:1], 2.0,
                        Alu.is_lt, Alu.mult,
                    )
                    nc.vector.tensor_tensor(
                        wq[:, k, c0:c1], a[:, c0:c1], b[:, c0:c1], Alu.subtract
                    )

            def quantize_pair(k):
                # batched dual-compares over two adjacent DVE k-tiles
                # (one 4096-col op each for a and b), subtracts per-tile
                a = mp_pool.tile([128, 2, OUT], bf16, tag="mpair", name=f"pa{k}")
                b = mp_pool.tile([128, 2, OUT], bf16, tag="mpair", name=f"pb{k}")
                nc.vector.tensor_scalar(
                    a[:], wh[:, k : k + 2, :], t_pos[:, 0:1], 2.0,
                    Alu.is_gt, Alu.mult,
                )
                nc.vector.tensor_scalar(
                    b[:], wh[:, k : k + 2, :], t_neg[:, 0:1], 2.0,
                    Alu.is_lt, Alu.mult,
                )
                for i in range(2):
                    nc.vector.tensor_tensor(
                        wq[:, k + i, :], a[:, i, :], b[:, i, :], Alu.subtract
                    )

            # k0 in column halves so the PE starts early.  Emission:
            # DVE-path tiles in sequence, ACT signs interleave (their
            # own queue), GpSimd adds its combines as signs complete.
            # prod carries modeled ready times to order phase-1.
            prod = []
            tD = 0.0
            tA = 0.2
            for h in range(2):
                quantize(0, h * 1024, (h + 1) * 1024)
                tD += 1.8
                prod.append((tD, 0, h * 1024, (h + 1) * 1024))
            for k in (1, 5, 9, 13):                     # DVE pairs (k, k+1)
                quantize_pair(k)
                tD += 4.6
                prod.append((tD - 1.4, k, 0, OUT))
                prod.append((tD, k + 1, 0, OUT))
            quantize(KT - 1, 0, OUT)
            tD += 2.6
            prod.append((tD, KT - 1, 0, OUT))
            for k in (3, 4, 7, 8, 11, 12):              # ACT-path tiles
                quantize(k, 0, OUT)
                tA += 4.1
                prod.append((tA + 2.5, k, 0, OUT))      # +accum-DMA latency
            prod.sort()

            # ---- phase 1: m0,m1 k-outer across 7 PSUM banks, consuming
            # wq slices in production order (k-accumulation commutes).
            # (m1,n3) is deferred so the pbc bank stays free for warm-
            # keeper dummies: a production stall that idles the PE
            # through a HAM window would re-gate it to 1.2 GHz and run
            # the stalled phase at double cost. ----
            P1_SET = [(m, n) for m in range(M_P1) for n in range(NT)]
            P1_SET.remove((1, NT - 1))
            p1 = {
                (m, n): psum_pool.tile([128, 512], f32, tag="psum", name=f"p1_{m}_{n}")
                for (m, n) in P1_SET
            }
            started = set()
            for idx, (_, k, c0, c1) in enumerate(prod):
                last = idx == len(prod) - 1
                for (m, n) in P1_SET:
                    lo, hi = n * 512, (n + 1) * 512
                    if hi <= c0 or lo >= c1:
                        continue
                    nc.tensor.matmul(
                        p1[(m, n)][:],
                        xbufs[m][:, k, :],
                        wq[:, k, lo:hi],
                        start=(m, n) not in started,
                        stop=last,
                    )
                    started.add((m, n))
                if idx in (5, 7, 9, 11, 13):
                    # warm-keeper: fills production stalls so the HAM
                    # clock gate never sees an idle window mid-phase
                    for _ in range(2):
                        nc.tensor.matmul(
                            pbc[:], wh[:, KT - 2, 0:128], wh[:, KT - 1, 0:512],
                            start=True, stop=True,
                        )

            def drain(m, n, psum, splits=1):
                st = stage_pool.tile([128, 512], f32, tag="stage", name=f"st{m}_{n}")
                w = 512 // splits
                for i in range(splits):
                    if splits > 1 and i == splits - 1:
                        # last piece on DVE so the two copies overlap
                        nc.vector.tensor_scalar(
                            st[:, i * w :], psum[:, i * w :], s_m[:, 0:1],
                            None, Alu.mult,
                        )
                    else:
                        nc.scalar.activation(
                            st[:, i * w : (i + 1) * w],
                            psum[:, i * w : (i + 1) * w],
                            Act.Copy, scale=s_m[:, 0:1],
                        )
                    nc.sync.dma_start(
                        out_ext[
                            m * 128 : (m + 1) * 128,
                            n * 512 + i * w : n * 512 + (i + 1) * w,
                        ],
                        st[:, i * w : (i + 1) * w],
                    )

            for (m, n) in P1_SET:
                drain(m, n, p1[(m, n)])

            # deferred (m1, n3): 16 clean matmuls overlapped with the
            # phase-1 drain copies above
            ps13 = psum_pool.tile([128, 512], f32, tag="psum", name="p1_late")
            for k in range(KT):
                nc.tensor.matmul(
                    ps13[:],
                    xbufs[1][:, k, :],
                    wq[:, k, (NT - 1) * 512 :],
                    start=(k == 0),
                    stop=(k == KT - 1),
                )
            drain(1, NT - 1, ps13)

            # ---- phase 2: m2..m7 n-outer / k-inner, per-n drains ----
            for m in range(M_P1, MT):
                for n in range(NT):
                    ps = psum_pool.tile(
                        [128, 512], f32, tag="psum", name=f"p2_{m}_{n}"
                    )
                    for k in range(KT):
                        nc.tensor.matmul(
                            ps[:],
                            xbufs[m][:, k, :],
                            wq[:, k, n * 512 : (n + 1) * 512],
                            start=(k == 0),
                            stop=(k == KT - 1),
                        )
                    last = m == MT - 1 and n == NT - 1
                    drain(m, n, ps, splits=2 if last else 1)

    nc.finalize()
    return nc


_NC_CACHE = None


def kernel(x, weight):
    global _NC_CACHE
    import ml_dtypes
    from concourse.bass_utils import run_bass_kernel_spmd

    x = np.asarray(x, dtype=np.float32).reshape(TOK, D)
    weight = np.asarray(weight, dtype=np.float32)
    wh = np.ascontiguousarray(weight.T).astype(np.float16)   # [in, out] fp16
    in_maps = []
    for i in range(N_CORES):
        shard_t = x[i * TPC : (i + 1) * TPC].T                      # [in, tok]
        tiled = (
            shard_t.reshape(KT, 128, MT, 128)
            .transpose(2, 1, 0, 3)
            .reshape(MT * 128, KT * 128)
        )
        in_maps.append(
            {"x": np.ascontiguousarray(tiled).astype(ml_dtypes.bfloat16),
             "wh": wh}
        )

    if _NC_CACHE is None:
        _NC_CACHE = build_kernel()
    res = run_bass_kernel_spmd(_NC_CACHE, in_maps, core_ids=list(range(N_CORES)))
    outs = [res.results[i]["out"] for i in range(N_CORES)]
    return np.concatenate(outs, axis=0).reshape(B, S, OUT).astype(np.float32)


# revision 45
# speedup vs baseline: 1.2345x; 1.2345x over previous
"""BitLinear (BitNet 1.58-bit ternary) distributed Trainium2 kernel.

Reference semantics:
    scale = max(mean(|w|), 1e-5)
    w_q   = sign(w) * (|w| > scale/3)          # ternary {-1, 0, 1}
    out   = (x @ w_q.T) * scale                # x: [4, 2048, 2048], w: [2048, 2048]

Sharding: data-parallel over tokens (1024 of 8192 per core), weight
replicated; each core computes the scale locally, so there are no
collectives (cross-core sync points absorb the harness' launch skew).

Host-side prep: transpose w to [in, out] and cast to fp16 — the ONLY
copy of w shipped (8 MiB/core instead of f32's 16+8), serving both the
scale reduction and the quantization.  fp16 rounding flips ~7e-5 of
the threshold comparisons (~0.9e-2 output rel err, gate is 2e-2).  x
is pre-cast to bf16 and pre-tiled m-major so every x DMA is contiguous
4KB-per-partition rows.

Pipeline (measured ~157us; baseline with f32-restream was ~209us):
  1. fp16 w streams once, 7x 1MiB + 3 smaller tail DMAs, on the SP
     HWDGE ring; |w| partial sums chase the stream, column-split
     between ACT (Abs + accum_out) and DVE (tensor_reduce absolute),
     whose combined throughput ~matches the ~460 GB/s arrival rate.
     x m-tiles queue behind w on the same FIFO ring — they land just
     before the matmul needs them without contending with the stream.
  2. scale: partials summed on DVE, broadcast to all 128 partitions
     with a ones-matmul into PSUM; thresholds +-t = +-scale/3 and the
     drain scale come straight off PSUM as dual-op tensor_scalars.
  3. quant, stored as 2*w_q (exact in bf16), split across engines at
     their measured rates (ACT sign 2.0us, DVE ~2.6us/tile for the
     3-op compare path):
       - 10 DVE-path tiles: 2[w>t] and -2[w<-t] via dual tensor_scalar
         (4x perf mode) + tensor_tensor subtract (2x);
       - 6 ACT-path tiles: Sign(w-t) written into wq, Sign(w+t) added
         by an SWDGE accumulate-DMA issued from the idle GpSimd queue
         (combine is off-engine; ~1.5 MiB SBUF traffic per tile rides
         unused fabric).
     The factor 2 folds into the drain scale (scale/2).
  4. matmul: bf16 PE, K contracted as 16 accumulating k-slices of 128.
     PSUM accumulation over k commutes, so phase 1 (m0,m1 over 7 PSUM
     banks, (m1,n3) deferred) consumes k-tiles in modeled production
     order, starting ~1.8us after the thresholds on a half-tile of k0.
     Warm-keeper dummy matmuls into the spare PSUM bank fill any
     production stall so the PE's HAM clock gate stays at 2.4 GHz (an
     idle 3.4us window would halve the clock and double the cost of
     the production-paced phase).  The deferred (m1,n3) pass then runs
     under the phase-1 drain copies, and phase 2 (m2..m7, n-outer /
     k-inner, 64 back-to-back matmuls per m-tile at the 216ns N=512
     roofline) streams out through per-n ACT copies (x scale/2) and
     output DMAs, the last one split so the final-byte tail is short.
"""
import sys

sys.path.insert(0, "/opt/trn_rl_repo")

import numpy as np

N_CORES = 8
B, S, D = 4, 2048, 2048        # x: [B, S, D]
OUT = 2048                     # out_features
TOK = B * S                    # 8192 tokens
TPC = TOK // N_CORES           # 1024 tokens per core
KT = D // 128                  # 16 K-tiles of 128
MT = TPC // 128                # 8 M-tiles per core
NT = OUT // 512                # 4 N-tiles of 512
N_ELEM = float(D * OUT)        # elements of w
EPS = 1e-5
M_P1 = 2                       # m-tiles in the k-outer first phase


def build_kernel():
    from concourse import bacc, tile, mybir
    from concourse.alu_op_type import AluOpType as Alu

    f32 = mybir.dt.float32
    bf16 = mybir.dt.bfloat16
    fp16 = mybir.dt.float16
    Act = mybir.ActivationFunctionType
    X = mybir.AxisListType.X

    nc = bacc.Bacc(None, target_bir_lowering=False)
    x_ext = nc.declare_dram_parameter("x", [TPC, D], bf16, isOutput=False)
    wh_ext = nc.declare_dram_parameter("wh", [D, OUT], fp16, isOutput=False)
    out_ext = nc.declare_dram_parameter("out", [TPC, OUT], f32, isOutput=True)

    with tile.TileContext(nc) as tc:
        with (
            tc.tile_pool(name="persist", bufs=1) as persist,
            tc.tile_pool(name="scr", bufs=1) as scr_pool,
            tc.tile_pool(name="sbuf2", bufs=5) as s_pool,
            tc.tile_pool(name="mbuf", bufs=3) as m_pool,
            tc.tile_pool(name="xbuf", bufs=8) as xbuf_pool,
            tc.tile_pool(name="stage", bufs=3) as stage_pool,
            tc.tile_pool(name="psum", bufs=8, space="PSUM") as psum_pool,
        ):
            wh = persist.tile([128, KT, OUT], fp16)      # resident fp16 w^T
            wq = persist.tile([128, KT, OUT], bf16)      # stored 2*w_q
            ones = persist.tile([128, 128], f32)
            partials = persist.tile([128, 30], f32)
            tot = persist.tile([128, 1], f32)
            tot2 = persist.tile([128, 1], f32)
            t_pos = persist.tile([128, 1], f32)
            t_neg = persist.tile([128, 1], f32)
            s_m = persist.tile([128, 1], f32)            # +scale/2
            scr = scr_pool.tile([128, OUT], fp16)        # |w| scratch (ACT Abs out)
            scr2 = scr_pool.tile([128, OUT], fp16)       # DVE accum scratch

            nc.vector.memset(ones[:], 1.0)
            nc.vector.memset(partials[:], 0.0)
            # pre-load the ACT function table so the first drain copy
            # doesn't pay ACT_TABLE_LOAD at phase-1 end
            tbl = persist.tile([128, 1], f32)
            nc.scalar.activation(tbl[:], ones[:, 0:1], Act.Copy)

            # ---- stream fp16 w: 7x 1MiB chunks + the last MiB as two
            # 0.5MiB DMAs (the second in two pieces) so the final
            # |w|-reduces start earlier.  ACT reduces most of the even
            # k-tile (Abs + accum_out), DVE its tail plus the odd tile
            # (tensor_reduce absolute) — an equal-lag column split at
            # the engines' measured rates. ----
            def reduce_pair(ke, ko, j):
                # ACT: 1968 cols of the even tile; DVE: its 80-col tail
                # plus the odd tile (equal-lag split at measured rates)
                nc.scalar.activation(
                    scr[:, 0:1968], wh[:, ke, 0:1968], Act.Abs,
                    accum_out=partials[:, j : j + 1],
                )
                nc.vector.tensor_reduce(
                    partials[:, 10 + j : 11 + j], wh[:, ke, 1968:],
                    axis=mybir.AxisListType.XY, op=Alu.add,
                    apply_absolute_value=True,
                )
                nc.vector.tensor_reduce(
                    partials[:, 20 + j : 21 + j], wh[:, ko, :],
                    axis=mybir.AxisListType.XY, op=Alu.add,
                    apply_absolute_value=True,
                )

            for j in range(KT // 2 - 1):
                nc.sync.dma_start(
                    wh[:, 2 * j : 2 * j + 2, :],
                    wh_ext[j * 256 : (j + 1) * 256, :].rearrange(
                        "(t p) o -> p t o", p=128
                    ),
                )
                reduce_pair(2 * j, 2 * j + 1, j)
            # last MiB as two half-chunks, each column-split across both
            nc.sync.dma_start(
                wh[:, KT - 2 : KT - 1, :],
                wh_ext[(KT - 2) * 128 : (KT - 1) * 128, :].rearrange(
                    "(t p) o -> p t o", p=128
                ),
            )
            nc.scalar.activation(
                scr[:, 0:1024], wh[:, KT - 2, 0:1024], Act.Abs,
                accum_out=partials[:, 7:8],
            )
            nc.vector.tensor_reduce(
                partials[:, 17:18], wh[:, KT - 2, 1024:],
                axis=mybir.AxisListType.XY, op=Alu.add,
                apply_absolute_value=True,
            )
            nc.sync.dma_start(
                wh[:, KT - 1 : KT, 0:1024],
                wh_ext[(KT - 1) * 128 :, 0:1024].rearrange(
                    "(t p) o -> p t o", p=128
                ),
            )
            nc.scalar.activation(
                scr[:, 0:1024], wh[:, KT - 1, 0:1024], Act.Abs,
                accum_out=partials[:, 8:9],
            )
            nc.sync.dma_start(
                wh[:, KT - 1 : KT, 1024:],
                wh_ext[(KT - 1) * 128 :, 1024:].rearrange(
                    "(t p) o -> p t o", p=128
                ),
            )
            nc.vector.tensor_reduce(
                partials[:, 18:19], wh[:, KT - 1, 1024:],
                axis=mybir.AxisListType.XY, op=Alu.add,
                apply_absolute_value=True,
            )

            # x m0/m1 queue on the SP ring behind the w stream, so they
            # land just before phase 1 without contending with it
            xbufs = {}

            def x_dma(m):
                xb = xbuf_pool.tile([128, KT, 128], bf16, tag="xbuf", name=f"xb{m}")
                nc.sync.dma_start(
                    xb[:],
                    x_ext[m * 128 : (m + 1) * 128, :].rearrange(
                        "p (k c) -> p k c", k=KT
                    ),
                )
                xbufs[m] = xb

            for m in range(MT):
                x_dma(m)

            # ---- PE warm-up: fp16 dummies gated on the last w chunk keep
            # the HAM busy so phase 1 starts at 2.4 GHz ----
            warm = psum_pool.tile([128, 512], f32, tag="psum", name="warm")
            for i in range(6):
                nc.tensor.matmul(
                    warm[:], wh[:, KT - 2, 0:128], wh[:, KT - 1, 0:512],
                    start=True, stop=True,
                )

            # ---- scale: sum partials, broadcast via ones-matmul ----
            nc.vector.tensor_reduce(tot[:], partials[:], axis=X, op=Alu.add)
            pbc = psum_pool.tile([128, 512], f32, tag="psum", name="pbc")
            nc.tensor.matmul(pbc[:, 0:1], ones[:], tot[:], start=True, stop=True)
            # keep PE busy through the scale->quant gap
            for i in range(4):
                nc.tensor.matmul(
                    warm[:], wh[:, KT - 2, 0:128], wh[:, KT - 1, 0:512],
                    start=True, stop=True,
                )
            nc.vector.tensor_scalar(
                t_pos[:], pbc[:, 0:1], 1.0 / (3.0 * N_ELEM), EPS / 3.0,
                Alu.mult, Alu.max,
            )
            nc.vector.tensor_scalar(
                t_neg[:], pbc[:, 0:1], -1.0 / (3.0 * N_ELEM), -EPS / 3.0,
                Alu.mult, Alu.min,
            )
            nc.vector.tensor_scalar(
                s_m[:], pbc[:, 0:1], 0.5 / N_ELEM, EPS / 2.0, Alu.mult, Alu.max,
            )

            # ---- quantize: stored wq = 2*w_q, split across all three
            # engines.  Odd k (ACT path): sign(w - t) + sign(w + t),
            # combined by GpSimd (k1..k9) or DVE (k11, k13); even k and
            # k15 (DVE path): 2[w > t] - 2[w < -t] via two dual
            # tensor_scalars (4x) and a tensor_tensor subtract (2x).
            # Drain scales by scale/2. ----
            def quantize(k, c0, c1):
                if k in (1, 3, 5, 7, 9, 11):
                    sn = s_pool.tile([128, OUT], bf16, tag="sbuf2", name=f"sn{k}")
                    nc.scalar.activation(
                        wq[:, k, c0:c1], wh[:, k, c0:c1], Act.Sign,
                        bias=t_neg[:, 0:1],
                    )
                    nc.scalar.activation(
                        sn[:, c0:c1], wh[:, k, c0:c1], Act.Sign, bias=t_pos[:, 0:1]
                    )
                    # combine off-engine: SWDGE accumulate-DMA adds sn into wq
                    nc.gpsimd.dma_start(
                        wq[:, k, c0:c1], sn[:, c0:c1], accum_op=Alu.add
                    )
                else:
                    a = m_pool.tile([128, OUT], bf16, tag="mbuf", name=f"a{k}_{c0}")
                    b = m_pool.tile([128, OUT], bf16, tag="mbuf", name=f"b{k}_{c0}")
                    nc.vector.tensor_scalar(
                        a[:, c0:c1], wh[:, k, c0:c1], t_pos[:, 0:1], 2.0,
                        Alu.is_gt, Alu.mult,
                    )
                    nc.vector.tensor_scalar(
                        b[:, c0:c1], wh[:, k, c0:c1], t_neg[:, 0:1], 2.0,
                        Alu.is_lt, Alu.mult,
                    )
                    nc.vector.tensor_tensor(
                        wq[:, k, c0:c1], a[:, c0:c1], b[:, c0:c1], Alu.subtract
                    )

            def quantize_pair(k):
                # batched dual-compares over two adjacent DVE k-tiles
                # (one 4096-col op each for a and b), subtracts per-tile
                a = m_pool.tile([128, 2, OUT], bf16, tag="mpair", name=f"pa{k}")
                b = m_pool.tile([128, 2, OUT], bf16, tag="mpair", name=f"pb{k}")
                nc.vector.tensor_scalar(
                    a[:], wh[:, k : k + 2, :], t_pos[:, 0:1], 2.0,
                    Alu.is_gt, Alu.mult,
                )
                nc.vector.tensor_scalar(
                    b[:], wh[:, k : k + 2, :], t_neg[:, 0:1], 2.0,
                    Alu.is_lt, Alu.mult,
                )
                for i in range(2):
                    nc.vector.tensor_tensor(
                        wq[:, k + i, :], a[:, i, :], b[:, i, :], Alu.subtract
                    )

            # k0 in column halves so the PE starts early.  Emission:
            # DVE-path tiles in sequence, ACT signs interleave (their
            # own queue), GpSimd adds its combines as signs complete.
            # prod carries modeled ready times to order phase-1.
            prod = []
            tD = 0.0
            tA = 0.2
            for h in range(2):
                quantize(0, h * 1024, (h + 1) * 1024)
                tD += 1.8
                prod.append((tD, 0, h * 1024, (h + 1) * 1024))
            for k in (2, 4, 6, 8, 10, 12, 14, 15, 13):  # DVE-path tiles
                quantize(k, 0, OUT)
                tD += 2.6
                prod.append((tD, k, 0, OUT))
            for k in (1, 3, 5, 7, 9, 11):               # ACT-path tiles
                quantize(k, 0, OUT)
                tA += 4.1
                prod.append((tA + 2.5, k, 0, OUT))      # +accum-DMA latency
            prod.sort()

            # ---- phase 1: m0,m1 k-outer across 7 PSUM banks, consuming
            # wq slices in production order (k-accumulation commutes).
            # (m1,n3) is deferred so the pbc bank stays free for warm-
            # keeper dummies: a production stall that idles the PE
            # through a HAM window would re-gate it to 1.2 GHz and run
            # the stalled phase at double cost. ----
            P1_SET = [(m, n) for m in range(M_P1) for n in range(NT)]
            P1_SET.remove((1, NT - 1))
            p1 = {
                (m, n): psum_pool.tile([128, 512], f32, tag="psum", name=f"p1_{m}_{n}")
                for (m, n) in P1_SET
            }
            started = set()
            for idx, (_, k, c0, c1) in enumerate(prod):
                last = idx == len(prod) - 1
                for (m, n) in P1_SET:
                    lo, hi = n * 512, (n + 1) * 512
                    if hi <= c0 or lo >= c1:
                        continue
                    nc.tensor.matmul(
                        p1[(m, n)][:],
                        xbufs[m][:, k, :],
                        wq[:, k, lo:hi],
                        start=(m, n) not in started,
                        stop=last,
                    )
                    started.add((m, n))
                if idx in (5, 7, 9, 11, 13):
                    # warm-keeper: fills production stalls so the HAM
                    # clock gate never sees an idle window mid-phase
                    for _ in range(2):
                        nc.tensor.matmul(
                            pbc[:], wh[:, KT - 2, 0:128], wh[:, KT - 1, 0:512],
                            start=True, stop=True,
                        )

            def drain(m, n, psum, splits=1):
                st = stage_pool.tile([128, 512], f32, tag="stage", name=f"st{m}_{n}")
                w = 512 // splits
                ring = nc.sync
                for i in range(splits):
                    nc.scalar.activation(
                        st[:, i * w : (i + 1) * w], psum[:, i * w : (i + 1) * w],
                        Act.Copy, scale=s_m[:, 0:1],
                    )
                    ring.dma_start(
                        out_ext[
                            m * 128 : (m + 1) * 128,
                            n * 512 + i * w : n * 512 + (i + 1) * w,
                        ],
                        st[:, i * w : (i + 1) * w],
                    )

            for (m, n) in P1_SET:
                drain(m, n, p1[(m, n)])

            # deferred (m1, n3): 16 clean matmuls overlapped with the
            # phase-1 drain copies above
            ps13 = psum_pool.tile([128, 512], f32, tag="psum", name="p1_late")
            for k in range(KT):
                nc.tensor.matmul(
                    ps13[:],
                    xbufs[1][:, k, :],
                    wq[:, k, (NT - 1) * 512 :],
                    start=(k == 0),
                    stop=(k == KT - 1),
                )
            drain(1, NT - 1, ps13)

            # ---- phase 2: m2..m7 n-outer / k-inner, per-n drains ----
            for m in range(M_P1, MT):
                for n in range(NT):
                    ps = psum_pool.tile(
                        [128, 512], f32, tag="psum", name=f"p2_{m}_{n}"
                    )
                    for k in range(KT):
                        nc.tensor.matmul(
                            ps[:],
                            xbufs[m][:, k, :],
                            wq[:, k, n * 512 : (n + 1) * 512],
                            start=(k == 0),
                            stop=(k == KT - 1),
                        )
                    last = m == MT - 1 and n == NT - 1
                    drain(m, n, ps, splits=2 if last else 1)

    nc.finalize()
    return nc


_NC_CACHE = None


def kernel(x, weight):
    global _NC_CACHE
    import ml_dtypes
    from concourse.bass_utils import run_bass_kernel_spmd

    x = np.asarray(x, dtype=np.float32).reshape(TOK, D)
    weight = np.asarray(weight, dtype=np.float32)
    wh = np.ascontiguousarray(weight.T).astype(np.float16)   # [in, out] fp16
    in_maps = []
    for i in range(N_CORES):
        shard_t = x[i * TPC : (i + 1) * TPC].T                      # [in, tok]
        tiled = (
            shard_t.reshape(KT, 128, MT, 128)
            .transpose(2, 1, 0, 3)
            .reshape(MT * 128, KT * 128)
        )
        in_maps.append(
            {"x": np.ascontiguousarray(tiled).astype(ml_dtypes.bfloat16),
             "wh": wh}
        )

    if _NC_CACHE is None:
        _NC_CACHE = build_kernel()
    res = run_bass_kernel_spmd(_NC_CACHE, in_maps, core_ids=list(range(N_CORES)))
    outs = [res.results[i]["out"] for i in range(N_CORES)]
    return np.concatenate(outs, axis=0).reshape(B, S, OUT).astype(np.float32)
